# revision 3
# baseline (speedup 1.0000x reference)
"""TRN2 Bass kernel for nn_MaxRollingMeanAttentionProbe (sparse_attention).

Computation (reference):
    y      = relu(x @ w1 + b1)                  # [n, 256]
    logits = y @ queries.T ; vals = y @ values.T  # [n, 16]
    per window i of size w: score_i = sum_j softmax(logits[i:i+w])_j * vals[i:i+w]_j
    out    = sum_h max_i score[i, h]            # scalar

Strategy: data-parallel over tokens across 8 NeuronCores with a recomputed
(w-1)-token halo (no collectives needed; the softmax shift cancels within any
window, so each partition uses its own local max shift).

Per core:
  pass A: stream host-pre-transposed x tiles [128 dm, 512 tok]; f32r matmuls
          accumulate yT = relu(w1.T @ xT + b1) in PSUM (full fp32 accumulate,
          inputs rounded to 11-bit mantissa = fp32r); probe matmuls produce
          logits/vals [16, 512]; staged to DRAM scratch [16, ntok].
  pass B: reload as [128 = 8 subchunks x 16 heads, 2048 + w - 1]; per-partition
          max-shift; exp on ScalarE; sliding-window sums of width w via log2(w)
          shifted adds on VectorE; scores = Wsum/Z; two-slice max-reduce
          -> [128, 2] per-core output.
Host: pack/round inputs, tiny final max/sum combine.
"""

import numpy as np

# Problem constants (shapes are fixed by the problem spec).
N_TOKENS = 131072
D_MODEL = 2048
D_HID = 256
N_HEADS = 16
N_CORES = 8
P = 128                    # SBUF partitions
G = 512                    # tokens per matmul/DMA group
TPC = N_TOKENS // N_CORES  # window starts per core (16384)
GPC = TPC // G             # groups per core without halo (32)
NSUB = 8                   # subchunks per core in pass B
SUB = TPC // NSUB          # window starts per subchunk (2048)
ND = D_MODEL // P          # 16 d_model chunks
NH2 = D_HID // P           # 2 hidden halves

_NC_CACHE = {}


def _round_fp32r(a: np.ndarray) -> np.ndarray:
    """Round-to-nearest-even to fp32r (11-bit mantissa), new array."""
    u = np.ascontiguousarray(a, dtype=np.float32).view(np.uint32)
    r = (u + np.uint32(0x800) + ((u >> np.uint32(12)) & np.uint32(1))) & np.uint32(
        0xFFFFF000
    )
    return r.view(np.float32)


def _window_sums(nc, a, b, L0, w, sub):
    """Sliding-window sums of width w over the free dim.

    a holds the window-1 values (valid length L0), b is a same-shape scratch.
    Returns the tile holding S_w with S_w[:, j] = sum_{u=j}^{j+w-1} a0[:, u],
    valid for j in [0, L0 - w + 1). Power-of-two w uses a log2(w) doubling
    chain; other w falls back to serial accumulation (correct, off the hot
    path — the problem instance uses w=64).
    """
    if w == 1:
        return a
    if (w & (w - 1)) == 0:
        cur, other = a, b
        p, L = 1, L0
        while p < w:
            nL = L - p
            nc.vector.tensor_add(
                out=other[:, 0:nL], in0=cur[:, 0:nL], in1=cur[:, p : p + nL]
            )
            cur, other = other, cur
            L = nL
            p *= 2
        return cur
    accL = L0 - w + 1
    nc.vector.tensor_add(out=b[:, 0:accL], in0=a[:, 0:accL], in1=a[:, 1 : 1 + accL])
    for k in range(2, w):
        nc.vector.tensor_add(
            out=b[:, 0:accL], in0=b[:, 0:accL], in1=a[:, k : k + accL]
        )
    return b


def _build(w: int):
    import concourse.bacc as bacc
    import concourse.tile as tile
    from concourse import mybir
    from contextlib import ExitStack

    F32 = mybir.dt.float32
    F32R = mybir.dt.float32r
    AF = mybir.ActivationFunctionType
    AX = mybir.AxisListType

    NG = -(-(TPC + w - 1) // G)    # groups per core incl. halo
    SUBLEN = SUB + w - 1           # tokens per subchunk
    SPLIT = SUB - w + 1            # j < SPLIT -> col 0; j >= SPLIT -> col 1
    TW = (SUBLEN + 15) // 16 * 16  # padded tile width

    nc = bacc.Bacc(
        "TRN2",
        target_bir_lowering=False,
        debug=False,
        enable_asserts=False,
        num_devices=N_CORES,
    )
    xg = nc.dram_tensor("xg", [NG, P, ND, G], F32R, kind="ExternalInput")
    w1p = nc.dram_tensor("w1p", [P, ND, D_HID], F32R, kind="ExternalInput")
    b1p = nc.dram_tensor("b1p", [P, NH2], F32, kind="ExternalInput")
    qTp = nc.dram_tensor("qTp", [P, NH2, N_HEADS], F32R, kind="ExternalInput")
    vTp = nc.dram_tensor("vTp", [P, NH2, N_HEADS], F32R, kind="ExternalInput")
    res = nc.dram_tensor("res", [P, 2], F32, kind="ExternalOutput")

    with tile.TileContext(nc) as tc, ExitStack() as ctx:
        const = ctx.enter_context(tc.tile_pool(name="const", bufs=1))
        w1_sb = const.tile([P, ND, D_HID], F32R)
        nc.sync.dma_start(out=w1_sb[:], in_=w1p[:])
        b1_sb = const.tile([P, NH2], F32)
        nc.sync.dma_start(out=b1_sb[:], in_=b1p[:])
        q_sb = const.tile([P, NH2, N_HEADS], F32R)
        nc.sync.dma_start(out=q_sb[:], in_=qTp[:])
        v_sb = const.tile([P, NH2, N_HEADS], F32R)
        nc.sync.dma_start(out=v_sb[:], in_=vTp[:])

        dram = ctx.enter_context(tc.tile_pool(name="dram", bufs=1, space="DRAM"))
        Lg = dram.tile([N_HEADS, NG * G], F32)
        Vg = dram.tile([N_HEADS, NG * G], F32)

        xpool = ctx.enter_context(tc.tile_pool(name="xpool", bufs=3))
        ypool = ctx.enter_context(tc.tile_pool(name="ypool", bufs=4))
        lvpool = ctx.enter_context(tc.tile_pool(name="lvpool", bufs=4))
        psy = ctx.enter_context(tc.tile_pool(name="psy", bufs=4, space="PSUM"))
        pslv = ctx.enter_context(tc.tile_pool(name="pslv", bufs=2, space="PSUM"))

        # ---------------- pass A: MLP + probes ----------------
        for g in range(NG):
            xt = xpool.tile([P, ND, G], F32R, tag="xt")
            nc.sync.dma_start(out=xt[:], in_=xg[g])
            yts = []
            for hh in range(NH2):
                ypt = psy.tile([P, G], F32, tag="ypsum")
                for d in range(ND):
                    nc.tensor.matmul(
                        ypt[:],
                        w1_sb[:, d, hh * P : (hh + 1) * P],
                        xt[:, d, :],
                        start=(d == 0),
                        stop=(d == ND - 1),
                    )
                yt = ypool.tile([P, G], F32R, tag="yt")
                nc.scalar.activation(
                    yt[:], ypt[:], AF.Relu, bias=b1_sb[:, hh : hh + 1]
                )
                yts.append(yt)
            lps = pslv.tile([N_HEADS, G], F32, tag="lps")
            vps = pslv.tile([N_HEADS, G], F32, tag="vps")
            for hh in range(NH2):
                nc.tensor.matmul(
                    lps[:], q_sb[:, hh, :], yts[hh][:],
                    start=(hh == 0), stop=(hh == NH2 - 1),
                )
            for hh in range(NH2):
                nc.tensor.matmul(
                    vps[:], v_sb[:, hh, :], yts[hh][:],
                    start=(hh == 0), stop=(hh == NH2 - 1),
                )
            ls = lvpool.tile([N_HEADS, G], F32, tag="ls")
            vs = lvpool.tile([N_HEADS, G], F32, tag="vs")
            nc.vector.tensor_copy(out=ls[:], in_=lps[:])
            nc.vector.tensor_copy(out=vs[:], in_=vps[:])
            nc.sync.dma_start(out=Lg[:, g * G : (g + 1) * G], in_=ls[:])
            nc.sync.dma_start(out=Vg[:, g * G : (g + 1) * G], in_=vs[:])

        # ---------------- pass B: windowed softmax-mean scores ----------------
        bp = ctx.enter_context(tc.tile_pool(name="bp", bufs=1))
        RL = bp.tile([P, TW], mybir.dt.float32)
        RV = bp.tile([P, TW], mybir.dt.float32)
        for s in range(NSUB):
            nc.sync.dma_start(
                out=RL[s * N_HEADS : (s + 1) * N_HEADS, 0:SUBLEN],
                in_=Lg[:, s * SUB : s * SUB + SUBLEN],
            )
            nc.sync.dma_start(
                out=RV[s * N_HEADS : (s + 1) * N_HEADS, 0:SUBLEN],
                in_=Vg[:, s * SUB : s * SUB + SUBLEN],
            )
        negm = bp.tile([P, 1], mybir.dt.float32)
        nc.vector.reduce_max(
            out=negm[:], in_=RL[:, 0:SUBLEN], axis=AX.X, negate=True
        )
        E = bp.tile([P, TW], mybir.dt.float32)
        nc.scalar.activation(E[:, 0:SUBLEN], RL[:, 0:SUBLEN], AF.Exp, bias=negm[:])
        EV = bp.tile([P, TW], mybir.dt.float32)
        nc.vector.tensor_mul(EV[:, 0:SUBLEN], E[:, 0:SUBLEN], RV[:, 0:SUBLEN])

        Z = _window_sums(nc, E, RL, SUBLEN, w, SUB)
        Wn = _window_sums(nc, EV, RV, SUBLEN, w, SUB)
        zfree = RL if Z is E else E    # scratch tiles not holding results
        wfree = RV if Wn is EV else EV

        nc.vector.reciprocal(out=zfree[:, 0:SUB], in_=Z[:, 0:SUB])
        nc.vector.tensor_mul(
            out=wfree[:, 0:SUB], in0=Wn[:, 0:SUB], in1=zfree[:, 0:SUB]
        )
        smax = bp.tile([P, 2], mybir.dt.float32)
        nc.vector.reduce_max(out=smax[:, 0:1], in_=wfree[:, 0:SPLIT], axis=AX.X)
        if SPLIT < SUB:
            nc.vector.reduce_max(
                out=smax[:, 1:2], in_=wfree[:, SPLIT:SUB], axis=AX.X
            )
        else:
            nc.vector.memset(smax[:, 1:2], -3.0e38)
        nc.sync.dma_start(out=res[:], in_=smax[:])

    nc.compile()
    return nc


def _get_nc(w: int):
    nc = _NC_CACHE.get(w)
    if nc is None:
        nc = _build(w)
        _NC_CACHE[w] = nc
    return nc


def _prep_inputs(x, w1, b1, queries, values, w):
    """Host-side packing: pad + fp32r-round + transpose into DMA-friendly
    layouts. Returns the per-core in_maps for run_bass_kernel_spmd."""
    NG = -(-(TPC + w - 1) // G)
    NGG = (N_CORES - 1) * GPC + NG  # distinct global groups incl. final halo
    xpad = np.zeros((NGG * G, D_MODEL), dtype=np.float32)
    xpad[:N_TOKENS] = x
    xr = _round_fp32r(xpad)
    # [gg, p, d, t] = xpad[gg*G + t, d*128 + p]
    xg_all = np.ascontiguousarray(
        xr.reshape(NGG, G, ND, P).transpose(0, 3, 2, 1)
    )
    w1p = np.ascontiguousarray(
        _round_fp32r(w1).reshape(ND, P, D_HID).transpose(1, 0, 2)
    )
    b1p = np.ascontiguousarray(np.asarray(b1, np.float32).reshape(NH2, P).T)
    qTp = np.ascontiguousarray(
        _round_fp32r(np.asarray(queries, np.float32).T)
        .reshape(NH2, P, N_HEADS)
        .transpose(1, 0, 2)
    )
    vTp = np.ascontiguousarray(
        _round_fp32r(np.asarray(values, np.float32).T)
        .reshape(NH2, P, N_HEADS)
        .transpose(1, 0, 2)
    )
    in_maps = []
    for c in range(N_CORES):
        in_maps.append(
            {
                "xg": xg_all[c * GPC : c * GPC + NG],
                "w1p": w1p,
                "b1p": b1p,
                "qTp": qTp,
                "vTp": vTp,
            }
        )
    return in_maps


def _combine(results, w):
    """Host-side final reduction: per-core [128, 2] -> scalar."""
    best = np.full(N_HEADS, -np.inf, dtype=np.float64)
    for c in range(N_CORES):
        r = np.asarray(results[c]["res"], dtype=np.float64).reshape(NSUB, N_HEADS, 2)
        if c == N_CORES - 1 and w >= 2:
            r = r.copy()
            r[NSUB - 1, :, 1] = -np.inf  # windows past n - w on the last core
        best = np.maximum(best, r.max(axis=(0, 2)))
    return np.asarray(best.sum(), dtype=np.float32)


def kernel(x, w1, b1, queries, values, window_size):
    from concourse.bass_utils import run_bass_kernel_spmd

    x = np.asarray(x, dtype=np.float32)
    w1 = np.asarray(w1, dtype=np.float32)
    b1 = np.asarray(b1, dtype=np.float32)
    queries = np.asarray(queries, dtype=np.float32)
    values = np.asarray(values, dtype=np.float32)
    w = int(np.asarray(window_size))
    assert x.shape == (N_TOKENS, D_MODEL), x.shape
    assert 1 <= w <= SUB

    nc = _get_nc(w)
    in_maps = _prep_inputs(x, w1, b1, queries, values, w)
    out = run_bass_kernel_spmd(nc, in_maps, core_ids=list(range(N_CORES)))
    return _combine(out.results, w)


# Optional: expose a traced run for profiling from test harnesses.
def kernel_traced(x, w1, b1, queries, values, window_size, tmpdir=None):
    from concourse.bass_utils import run_bass_kernel_spmd

    w = int(np.asarray(window_size))
    nc = _get_nc(w)
    in_maps = _prep_inputs(
        np.asarray(x, np.float32),
        np.asarray(w1, np.float32),
        np.asarray(b1, np.float32),
        np.asarray(queries, np.float32),
        np.asarray(values, np.float32),
        w,
    )
    out = run_bass_kernel_spmd(
        nc, in_maps, core_ids=list(range(N_CORES)), trace=True, tmpdir=tmpdir
    )
    return _combine(out.results, w), out
